# revision 6
# baseline (speedup 1.0000x reference)
"""Trainium2 Bass kernel for ContrastiveMSELoss (v12).

Loss collapses to class-bucketed moments:
    T_same = (2 sum_i n_{c_i} |x_i|^2 - 2 sum_c |M_c|^2) / D
    T_all  = (2 N sum_i |x_i|^2 - 2 |M|^2) / D
    loss   = (2 T_same - T_all) / N^2 + BETA

v7 tail re-plumb (vs v5, all trace-driven):
  - the PSUM->SBUF copy takes its `scale` operand from a [128,1] "ones"
    tile produced from the last ACT square's accumulator, forcing the
    tile scheduler to keep the copy AFTER both ACT squares (in v5/v6 it
    hoisted the copy between them, delaying the bn output by ~1us)
  - chunks 6-7 ride the scalar queue behind chunks 3-5 so the sync
    queue finishes earlier and the matmul chain stops sooner
  - one-hot runs on Pool so DVE starts its bn_stats chain immediately
  - stats out is issued from scalar (cheaper issue than sync), bn out
    from sync; the two outputs travel on separate queues in parallel

Device outputs per core:
  - stats [40, 256] bf16: per-class sums M_c (one-hot matmul chain)
  - bno [128, 8, 6] f32: per-row second-moment stats (bn_stats on DVE
    for 6 chunks, Square+accumulator on ACT for 2); host folds
        sq = Y2 + Y5 + 128*(Y1^2 + Y4^2)
    and combines everything in float64.
"""

import numpy as np
import ml_dtypes

import concourse.bacc as bacc
import concourse.bass as bass
import concourse.tile as tile
from concourse import mybir
from concourse.bass_utils import run_bass_kernel_spmd

N, D = 8192, 256
N_CORES = 8
ROWS = N // N_CORES          # 1024 rows per core
P = 128                      # partitions
CHUNKS = ROWS // P           # 8 chunks of 128 rows
NCLS = 40
BETA = 1.0
COMBO = NCLS + CHUNKS        # 48 header cols: iota | cls
W = COMBO + CHUNKS * D       # 2096 packed cols
WARMUP_MM = 6
BNF = 6                      # bn_stats fields per chunk
ACT_CHUNKS = (4, 5)          # squared on ACT (earliest slab); rest DVE bn_stats

_CACHE = {}


def _bcast(ap, pos, count):
    """Insert a zero-stride dim of size `count` at free-dim position `pos`."""
    pattern = [list(p) for p in ap.ap]
    pattern.insert(pos, [0, count])
    return bass.AP(tensor=ap.tensor, offset=ap.offset, ap=pattern)


def _build_bass():
    nc = bacc.Bacc(
        "TRN2",
        target_bir_lowering=False,
        debug=False,
        enable_asserts=True,
        num_devices=N_CORES,
    )
    f32 = mybir.dt.float32
    bf16 = mybir.dt.bfloat16

    x = nc.dram_tensor("x", [P, W], bf16, kind="ExternalInput")
    stats = nc.dram_tensor("stats", [NCLS, D], bf16, kind="ExternalOutput")
    bno = nc.dram_tensor("bn", [P, CHUNKS * BNF + 1], f32, kind="ExternalOutput")

    with tile.TileContext(nc) as tc:
        with (
            tc.tile_pool(name="work", bufs=1) as work,
            tc.tile_pool(name="psum", bufs=1, space="PSUM") as psum_pool,
        ):
            xin = work.tile([P, W], bf16, tag="xin")
            oh = work.tile([P, CHUNKS, NCLS], bf16, tag="oh")
            bnt = work.tile([P, CHUNKS * BNF + 1], f32, tag="bnt")
            scra = work.tile([P, D], bf16, tag="scra")
            one = work.tile([P, 1], f32, tag="one")
            ost = work.tile([NCLS, D], bf16, tag="ost")
            jw = work.tile([P, D], bf16, tag="jw")
            acc = psum_pool.tile([NCLS, D], f32, tag="acc")
            jacc = psum_pool.tile([NCLS, D], f32, tag="jacc")

            # pool: zero the stats buffer (ACT chunks only write field 2)
            # and feed the PE warmup chain.
            nc.gpsimd.memset(bnt[:, :], 0.0)
            nc.gpsimd.memset(jw[:, :], 0.25)
            for _ in range(WARMUP_MM):
                nc.tensor.matmul(jacc, jw[:, 0:NCLS], jw[:, :], start=True, stop=True)

            def col(k):
                return COMBO + k * D

            # input DMAs: sync carries header + chunks 0-2 (and later the bn
            # output); scalar carries chunks 3-5 then 6-7 (and the stats out).
            nc.sync.dma_start(out=xin[:, 0:COMBO], in_=x[:, 0:COMBO])
            nc.scalar.dma_start(out=xin[:, col(3):col(6)], in_=x[:, col(3):col(6)])
            nc.sync.dma_start(out=xin[:, col(0):col(3)], in_=x[:, col(0):col(3)])
            nc.scalar.dma_start(out=xin[:, col(6):col(8)], in_=x[:, col(6):col(8)])

            iota_sb = xin[:, 0:NCLS]
            cls_sb = xin[:, NCLS:COMBO]

            # one-hot: oh[p, k, c] = (cls[p, k] == c); only gated on the tiny
            # header DMA (Pool rejects the broadcast APs, so it runs on DVE).
            nc.vector.tensor_tensor(
                out=oh[:, :, :],
                in0=_bcast(cls_sb, 2, NCLS),
                in1=_bcast(iota_sb, 1, CHUNKS),
                op=mybir.AluOpType.is_equal,
            )

            # row second moments
            for k in ACT_CHUNKS:
                nc.scalar.activation(
                    out=scra,
                    in_=xin[:, col(k):col(k + 1)],
                    func=mybir.ActivationFunctionType.Square,
                    accum_out=bnt[:, k * BNF + 2:k * BNF + 3],
                )
            for k in (3, 0, 1, 2, 7, 6):
                nc.vector.bn_stats(bnt[:, k * BNF:(k + 1) * BNF], xin[:, col(k):col(k + 1)])

            # per-class sums M_c: one matmul per chunk, arrival order
            order = [3, 4, 5, 0, 1, 2, 6, 7]
            for i, k in enumerate(order):
                nc.tensor.matmul(
                    acc,
                    oh[:, k, :],
                    xin[:, col(k):col(k + 1)],
                    start=(i == 0),
                    stop=(i == CHUNKS - 1),
                )

            # ones tile derived from the last ACT square's accumulator: a pure
            # scheduling dependency so the copy below cannot be hoisted
            # between the ACT squares (x == x is 1.0 for finite x).
            # ones tile = last_acc*0 + 1 on the idle Pool engine (DVE is
            # busy with bn_stats until after the matmul chain stops, and the
            # scheduler always pushed a DVE ones-op to the end of its stream)
            last_acc = bnt[:, ACT_CHUNKS[-1] * BNF + 2:ACT_CHUNKS[-1] * BNF + 3]
            nc.gpsimd.tensor_scalar(
                out=one,
                in0=last_acc,
                scalar1=0.0,
                scalar2=1.0,
                op0=mybir.AluOpType.mult,
                op1=mybir.AluOpType.add,
            )

            # PSUM -> SBUF on ACT, scaled by the (exactly 1.0) ones tile
            nc.scalar.activation(
                out=ost,
                in_=acc[:, :],
                func=mybir.ActivationFunctionType.Copy,
                scale=one[0:NCLS, 0:1],
            )
            # pin the bn DMA after the copy in the scalar stream: a tiny ACT
            # op reads the copy's output into the bn buffer's junk column, so
            # the scheduler cannot hoist the bn issue above the copy. The
            # stats DMA then rides the idle sync queue (whose issues are
            # ~450ns cheaper than scalar's).
            nc.scalar.activation(
                out=bnt[0:NCLS, CHUNKS * BNF:CHUNKS * BNF + 1],
                in_=ost[:, 0:1],
                func=mybir.ActivationFunctionType.Copy,
            )
            nc.sync.dma_start(out=stats[:, :], in_=ost)
            nc.scalar.dma_start(out=bno[:, :], in_=bnt[:, :])

    return nc


def _get_nc():
    if "nc" not in _CACHE:
        nc = _build_bass()
        nc.finalize()
        _CACHE["nc"] = nc
    return _CACHE["nc"]


_IOTA = np.broadcast_to(
    np.arange(NCLS, dtype=np.float32).astype(ml_dtypes.bfloat16), (P, NCLS)
)


def run_device(output, classes, **spmd_kwargs):
    """Run the per-core Bass kernel; returns (per-core (stats, bn), results)."""
    x = np.asarray(output).astype(ml_dtypes.bfloat16)
    cls_f = np.asarray(classes).astype(np.float32).astype(ml_dtypes.bfloat16)
    in_maps = []
    for s in range(N_CORES):
        xs = x[s * ROWS:(s + 1) * ROWS]
        cs = cls_f[s * ROWS:(s + 1) * ROWS]
        packed = np.empty((P, W), dtype=ml_dtypes.bfloat16)
        packed[:, 0:NCLS] = _IOTA
        packed[:, NCLS:COMBO] = cs.reshape(CHUNKS, P).T
        packed[:, COMBO:] = (
            xs.reshape(CHUNKS, P, D).transpose(1, 0, 2).reshape(P, CHUNKS * D)
        )
        in_maps.append({"x": packed})
    res = run_bass_kernel_spmd(
        _get_nc(), in_maps, core_ids=list(range(N_CORES)), **spmd_kwargs
    )
    outs = [(res.results[s]["stats"], res.results[s]["bn"]) for s in range(N_CORES)]
    return outs, res


def _combine(outs, classes):
    """Combine per-core partial outputs into the scalar loss (float64)."""
    cls = np.asarray(classes).astype(np.int64)
    M_c = np.zeros((NCLS, D), dtype=np.float64)
    sq_full = np.empty(N, dtype=np.float64)
    for s, (stats, bn) in enumerate(outs):
        M_c += np.asarray(stats).astype(np.float64)
        y = np.asarray(bn)[:, :CHUNKS * BNF].astype(np.float64).reshape(P, CHUNKS, BNF)
        sq = y[:, :, 2] + y[:, :, 5] + 128.0 * (y[:, :, 1] ** 2 + y[:, :, 4] ** 2)
        # sq[p, k] holds shard row k*128+p
        sq_full[s * ROWS:(s + 1) * ROWS] = sq.T.reshape(-1)
    n_c = np.bincount(cls, minlength=NCLS).astype(np.float64)
    SQ = sq_full.sum()
    S1 = (n_c[cls] * sq_full).sum()
    M = M_c.sum(axis=0)
    T_same = (2.0 * S1 - 2.0 * (M_c * M_c).sum()) / D
    T_all = (2.0 * N * SQ - 2.0 * (M @ M)) / D
    loss = (2.0 * T_same - T_all) / (float(N) * float(N)) + BETA
    return np.float32(loss)


def kernel(output, classes):
    outs, _ = run_device(output, classes)
    return _combine(outs, classes)
